# revision 18
# baseline (speedup 1.0000x reference)
"""MQA (GQA with 1 KV group) attention kernel for 8 Trainium2 NeuronCores.

Sharding: core c -> batch b = c//4, head-group hg = c%4 (4 of 16 query heads).
Each core computes Q/K/V projections from x[b]^T, causal attention for its 4
heads in transposed layout (S^T[kv, q] tiles), and a partial output
projection out_partial = A_h @ Wo[:, cols_h]^T.  Host sums the 4 partials per
batch and adds bo.

v2 notes (vs the original): attention runs as two 2-head streams with PSUM
ping-pong (2 banks scores + 2 banks AV accum per stream = 8 banks total) and
a one-step software pipeline (AV of kt-1 issued with scores of kt) so the PE
never waits on the exp activation; softmax row-sums accumulate on the DVE in
bf16 and are broadcast-reduced by an all-ones matmul, then inverted with the
single-pass reciprocal_approx_fast; the causal mask is structural (upper
kv-tiles skipped, diagonal-band es tiles pre-zeroed and exp restricted to
valid columns, only the 128-wide triangle gets a mask multiply).
"""

import sys

sys.path.insert(0, "/opt/trn_rl_repo")

import ml_dtypes
import numpy as np

import concourse.bass as bass
import concourse.tile as tile
from concourse import bacc
from concourse import mybir
from concourse.bass import ts
from concourse.bass_utils import run_bass_kernel_spmd
from concourse.masks import make_identity

B, S, HID = 2, 2048, 2048
H, D = 16, 128
HPC = 4              # heads per core
DPH = HPC * D        # 512: head dims per core
NCORES = 8
SC = 512             # s-chunk (free dim for most matmuls)
NSC = S // SC        # 4
NT = S // 128        # 16 128-tiles along s / hid
NHT = HID // 128     # 16 hid tiles
SCALE = 1.0 / float(np.sqrt(D))
NEG = -1.0e9

F32 = mybir.dt.float32
BF16 = mybir.dt.bfloat16
NP_BF16 = ml_dtypes.bfloat16

_PROGRAM = None
LAST_RESULT = None


def _build_program():
    nc = bacc.Bacc()
    xT = nc.declare_dram_parameter("xT", [HID, S], BF16, isOutput=False)
    wq = nc.declare_dram_parameter("wq", [HID, DPH], BF16, isOutput=False)
    wk = nc.declare_dram_parameter("wk", [HID, D], BF16, isOutput=False)
    wv = nc.declare_dram_parameter("wv", [HID, D], BF16, isOutput=False)
    wo = nc.declare_dram_parameter("wo", [DPH, HID], BF16, isOutput=False)
    bq = nc.declare_dram_parameter("bq", [128, HPC], F32, isOutput=False)
    bkv = nc.declare_dram_parameter("bkv", [128, 2], F32, isOutput=False)
    padb = nc.declare_dram_parameter("padb", [128, NT], F32, isOutput=False)
    tri = nc.declare_dram_parameter("tri", [128, 2, 128], BF16, isOutput=False)
    out = nc.declare_dram_parameter("out", [S, HID], BF16, isOutput=True)

    Exp = mybir.ActivationFunctionType.Exp
    Ident = mybir.ActivationFunctionType.Identity

    with tile.TileContext(nc) as tc:
        with (
            tc.tile_pool(name="consts", bufs=1) as consts,
            tc.tile_pool(name="persist", bufs=1) as persist,
        ):
            # consts ride the GpSimd DGE so the Sync queue is free to start
            # streaming xT rows immediately
            ident = consts.tile([128, 128], BF16)
            make_identity(nc, ident[:])
            ones128 = consts.tile([128, 128], BF16)
            nc.vector.memset(ones128[:], 1.0)
            bq_sb = consts.tile([128, HPC], F32)
            nc.gpsimd.dma_start(bq_sb[:], bq[:])
            bkv_sb = consts.tile([128, 2], F32)
            nc.gpsimd.dma_start(bkv_sb[:], bkv[:])
            padb_sb = consts.tile([128, NT], F32)
            nc.gpsimd.dma_start(padb_sb[:], padb[:])
            tri_sb = consts.tile([128, 2, 128], BF16)
            nc.gpsimd.dma_start(tri_sb[:], tri[:])

            # Persistent activations (live across stages)
            wo_sb = persist.tile([128, HPC, HID], BF16)  # stage-3 weights
            QT = persist.tile([128, HPC, S], BF16)   # Q^T per head: [d, h, q]
            KT = persist.tile([128, S], BF16)        # K^T: [d, kv]
            V = persist.tile([128, NT, 128], BF16)   # V tiles: [kv_p, kv_tile, d]
            OT = persist.tile([128, HPC, S], BF16)   # normalized (exp(S) V)^T

            # Diagonal-band es tiles: columns left of the diagonal stay zero
            # forever (exp never writes them), so AV/rowsum reads are exact.
            esd = [
                [
                    persist.tile([128, 2, SC], BF16, name=f"esd_{j}_{s}")
                    for s in range(2)
                ]
                for j in range(4)
            ]
            for j in range(4):
                for s in range(2):
                    nc.vector.memset(esd[j][s][:], 0.0)

            # ---------------- Stage 1: projections ----------------
            with (
                tc.tile_pool(name="w1", bufs=1) as w1p,
                tc.tile_pool(name="xr", bufs=1) as xrp,
                tc.tile_pool(name="vt", bufs=2) as vtp,
                tc.tile_pool(name="ps1", bufs=1, space="PSUM") as ps1,
                tc.tile_pool(name="pstr", bufs=2, space="PSUM") as pstr,
            ):
                # x^T resident: one whole-row DMA per 128-partition tile
                # (4 KiB/partition contiguous descriptors) on the Sync DGE;
                # weights ride the Scalar DGE in parallel.
                xres = []
                for ht in range(NHT):
                    xr_t = xrp.tile([128, S], BF16, tag=f"x{ht}",
                                    name=f"xr_{ht}")
                    nc.sync.dma_start(xr_t[:], xT[ts(ht, 128), :])
                    xres.append(xr_t)
                wk_sb = w1p.tile([128, NHT, D], BF16)
                nc.scalar.dma_start(
                    wk_sb[:], wk.rearrange("(t p) d -> p t d", p=128)
                )
                wv_sb = w1p.tile([128, NHT, D], BF16)
                nc.scalar.dma_start(
                    wv_sb[:], wv.rearrange("(t p) d -> p t d", p=128)
                )
                wq_sb = w1p.tile([128, NHT, DPH], BF16)
                nc.scalar.dma_start(
                    wq_sb[:], wq.rearrange("(t p) d -> p t d", p=128)
                )

                for sc in range(NSC):
                    xts = [xres[ht][:, ts(sc, SC)] for ht in range(NHT)]
                    # K^T chunk
                    psk = ps1.tile([128, SC], F32, tag="k")
                    for ht in range(NHT):
                        nc.tensor.matmul(
                            psk[:], wk_sb[:, ht, :], xts[ht],
                            start=(ht == 0), stop=(ht == NHT - 1),
                        )
                    nc.scalar.activation(
                        KT[:, ts(sc, SC)], psk[:], Ident, bias=bkv_sb[:, 0:1]
                    )
                    # V^T chunk -> transpose into V tiles
                    psv = ps1.tile([128, SC], F32, tag="v")
                    for ht in range(NHT):
                        nc.tensor.matmul(
                            psv[:], wv_sb[:, ht, :], xts[ht],
                            start=(ht == 0), stop=(ht == NHT - 1),
                        )
                    vt_s = vtp.tile([128, SC], BF16, tag="vt")
                    nc.scalar.activation(
                        vt_s[:], psv[:], Ident, bias=bkv_sb[:, 1:2]
                    )
                    for j in range(SC // 128):
                        pst = pstr.tile([128, 128], BF16, tag="tr")
                        nc.tensor.transpose(pst[:], vt_s[:, ts(j, 128)], ident[:])
                        nc.scalar.copy(V[:, sc * 4 + j, :], pst[:])
                    # Q^T chunks (4 heads)
                    for dt in range(HPC):
                        psq = ps1.tile([128, SC], F32, tag=f"q{dt}")
                        for ht in range(NHT):
                            nc.tensor.matmul(
                                psq[:], wq_sb[:, ht, ts(dt, 128)], xts[ht],
                                start=(ht == 0), stop=(ht == NHT - 1),
                            )
                        nc.scalar.activation(
                            QT[:, dt, ts(sc, SC)], psq[:], Ident,
                            bias=bq_sb[:, dt : dt + 1],
                        )

            # ---------------- Stage 2: attention ----------------
            # Two 2-head streams s=0 (heads 0,1) and s=1 (heads 2,3).
            with (
                tc.tile_pool(name="es", bufs=6) as esp,
                tc.tile_pool(name="acc", bufs=2) as accp,
                tc.tile_pool(name="rb", bufs=2) as rbp,
                tc.tile_pool(name="psSa", bufs=1, space="PSUM") as psSa,
                tc.tile_pool(name="psSb", bufs=1, space="PSUM") as psSb,
                tc.tile_pool(name="psOa", bufs=1, space="PSUM") as psOa,
                tc.tile_pool(name="psOb", bufs=1, space="PSUM") as psOb,
            ):
                # prefetch the stage-3 weights while the DMA queue is idle
                nc.sync.dma_start(
                    wo_sb[:], wo.rearrange("(t p) d -> p t d", p=128)
                )
                psS_pool = [psSa, psSb]
                psO_pool = [psOa, psOb]
                pending_fin = None

                def emit_finalize():
                    # normalize: rowsum broadcast via all-ones matmul,
                    # single-pass approx reciprocal, multiply into OT
                    nonlocal pending_fin
                    if pending_fin is None:
                        return
                    fqc, fpsos, faccs = pending_fin
                    pending_fin = None
                    for s in range(2):
                        psb = psS_pool[s].tile(
                            [128, 2, SC], F32, tag="s", name=f"psb_{s}"
                        )
                        for h in range(2):
                            nc.tensor.matmul(
                                psb[:, h, :], ones128[:], faccs[s][:, h, :],
                                start=True, stop=True,
                            )
                        rb = rbp.tile([128, 2, SC], F32, tag=f"r{s}")
                        nc.vector.reciprocal_approx_fast(rb[:], psb[:])
                        nc.vector.tensor_mul(
                            OT[:, 2 * s : 2 * s + 2, ts(fqc, SC)],
                            fpsos[s][:], rb[:],
                        )

                for qc in range(NSC):
                    nkt = 4 * qc + 4
                    psos = [
                        psO_pool[s].tile(
                            [128, 2, SC], F32, tag="o", name=f"pso_{s}"
                        )
                        for s in range(2)
                    ]
                    accs = [
                        accp.tile(
                            [128, 2, SC], BF16, tag=f"a{s}", name=f"acc_{s}"
                        )
                        for s in range(2)
                    ]
                    es_hist = {}

                    def emit_av(kt, psos=psos, es_hist=es_hist, nkt=nkt,
                                qc=qc):
                        j = kt - 4 * qc
                        lo = 128 * j if j >= 0 else 0
                        for s in range(2):
                            es_t = es_hist[(kt, s)]
                            for h in range(2):
                                nc.tensor.matmul(
                                    psos[s][:, h, lo:SC], V[:, kt, :],
                                    es_t[:, h, lo:SC],
                                    start=(kt == 0), stop=(kt == nkt - 1),
                                    skip_group_check=(j > 0),
                                )

                    for kt in range(nkt):
                        j = kt - 4 * qc
                        lo = 128 * j if j >= 0 else 0
                        # scores matmuls (diagonal kts: only valid columns)
                        pss = [
                            psS_pool[s].tile(
                                [128, 2, SC], F32, tag="s", name=f"pss_{s}"
                            )
                            for s in range(2)
                        ]
                        for s in range(2):
                            for h in range(2):
                                nc.tensor.matmul(
                                    pss[s][:, h, lo:SC], KT[:, ts(kt, 128)],
                                    QT[:, 2 * s + h,
                                       qc * SC + lo : (qc + 1) * SC],
                                    start=True, stop=True,
                                )
                        # exp (+ padding bias); diagonal kts write only the
                        # valid column range of the pre-zeroed esd tiles
                        for s in range(2):
                            if j >= 0:
                                es_t = esd[j][s]
                                nc.scalar.activation(
                                    es_t[:, :, lo:SC], pss[s][:, :, lo:SC],
                                    Exp, bias=padb_sb[:, kt : kt + 1],
                                    scale=SCALE,
                                )
                            else:
                                es_t = esp.tile(
                                    [128, 2, SC], BF16, tag=f"e{s}",
                                    name=f"es_{s}",
                                )
                                nc.scalar.activation(
                                    es_t[:], pss[s][:], Exp,
                                    bias=padb_sb[:, kt : kt + 1], scale=SCALE,
                                )
                            es_hist[(kt, s)] = es_t
                        # triangle mask on the 128-wide diagonal block
                        if j >= 0:
                            for s in range(2):
                                es_t = es_hist[(kt, s)]
                                nc.vector.tensor_mul(
                                    es_t[:, :, lo : lo + 128],
                                    es_t[:, :, lo : lo + 128],
                                    tri_sb[:],
                                )
                        # row-sum accumulation (bf16, 2x DVE rate)
                        for s in range(2):
                            es_t = es_hist[(kt, s)]
                            if kt == 0:
                                nc.vector.tensor_scalar_mul(
                                    accs[s][:], es_t[:], 1.0
                                )
                            else:
                                nc.vector.tensor_add(
                                    accs[s][:], accs[s][:], es_t[:]
                                )
                        # AV of previous kt (software pipeline: PE never
                        # waits on this kt's exp); previous qc's finalize
                        # slots in after the first scores of this qc
                        if kt > 0:
                            emit_av(kt - 1)
                        if kt == 0:
                            emit_finalize()
                    emit_av(nkt - 1)
                    pending_fin = (qc, psos, accs)
                emit_finalize()

            # ---------------- Stage 3: output projection ----------------
            with (
                tc.tile_pool(name="outsb", bufs=4) as outp,
                tc.tile_pool(name="ps3", bufs=2, space="PSUM") as ps3,
            ):
                for st in range(NT):
                    psT = ps3.tile([128, HID // SC, SC], F32, tag="c")
                    for dt in range(HPC):
                        for hc in range(HID // SC):
                            nc.tensor.matmul(
                                psT[:, hc, :],
                                OT[:, dt, ts(st, 128)],
                                wo_sb[:, dt, ts(hc, SC)],
                                start=(dt == 0), stop=(dt == HPC - 1),
                            )
                    # evacuate: half on ACT, half on DVE; 4 output DMAs
                    o1 = outp.tile([128, 2, SC], BF16, tag="o1")
                    nc.scalar.copy(o1[:], psT[:, 0:2, :])
                    o2 = outp.tile([128, 2, SC], BF16, tag="o2")
                    nc.vector.tensor_scalar_mul(o2[:], psT[:, 2:4, :], 1.0)
                    for hc in range(2):
                        nc.sync.dma_start(
                            out[ts(st, 128), ts(hc, SC)], o1[:, hc, :]
                        )
                        nc.sync.dma_start(
                            out[ts(st, 128), ts(hc + 2, SC)], o2[:, hc, :]
                        )
    nc.compile()
    return nc


def _get_program():
    global _PROGRAM
    if _PROGRAM is None:
        _PROGRAM = _build_program()
    return _PROGRAM


def kernel(**inputs):
    global LAST_RESULT
    hs = np.ascontiguousarray(inputs["hidden_states"], dtype=np.float32)
    pad = np.ascontiguousarray(inputs["padding_mask"], dtype=np.float32)
    Wq = np.asarray(inputs["Wq"], dtype=np.float32)
    Wk = np.asarray(inputs["Wk"], dtype=np.float32)
    Wv = np.asarray(inputs["Wv"], dtype=np.float32)
    Wo = np.asarray(inputs["Wo"], dtype=np.float32)
    bq_v = np.asarray(inputs["bq"], dtype=np.float32)
    bk_v = np.asarray(inputs["bk"], dtype=np.float32)
    bv_v = np.asarray(inputs["bv"], dtype=np.float32)
    bo_v = np.asarray(inputs["bo"], dtype=np.float32)

    xTs = [np.ascontiguousarray(hs[b].T).astype(NP_BF16) for b in range(B)]
    WqT = Wq.T  # [HID, HID]
    WkT = np.ascontiguousarray(Wk.T).astype(NP_BF16)  # [HID, D]
    WvT = np.ascontiguousarray(Wv.T).astype(NP_BF16)
    WoT = Wo.T  # [HID, HID]

    # triangle mask for the 128-wide diagonal block: tri[p, q] = 1 if p <= q
    p_i = np.arange(128)[:, None]
    f_i = np.arange(128)[None, :]
    tri1 = (p_i <= f_i).astype(np.float32)
    tri = np.ascontiguousarray(
        np.broadcast_to(tri1[:, None, :], (128, 2, 128))
    ).astype(NP_BF16)

    padbs = [
        np.ascontiguousarray((NEG * pad[b]).reshape(NT, 128).T) for b in range(B)
    ]
    bqs = [
        np.ascontiguousarray(
            bq_v[hg * DPH : (hg + 1) * DPH].reshape(HPC, 128).T
        )
        for hg in range(HPC)
    ]
    bkv = np.ascontiguousarray(np.stack([bk_v, bv_v], axis=1))  # [128, 2]

    nc = _get_program()
    in_maps = []
    for c in range(NCORES):
        b, hg = c // 4, c % 4
        in_maps.append(
            {
                "xT": xTs[b],
                "wq": np.ascontiguousarray(
                    WqT[:, hg * DPH : (hg + 1) * DPH]
                ).astype(NP_BF16),
                "wk": WkT,
                "wv": WvT,
                "wo": np.ascontiguousarray(
                    WoT[hg * DPH : (hg + 1) * DPH, :]
                ).astype(NP_BF16),
                "bq": bqs[hg],
                "bkv": bkv,
                "padb": padbs[b],
                "tri": tri,
            }
        )

    LAST_RESULT = run_bass_kernel_spmd(nc, in_maps, list(range(NCORES)))
    res = LAST_RESULT.results

    outp = np.zeros((B, S, HID), np.float32)
    for c in range(NCORES):
        outp[c // 4] += res[c]["out"]
    outp += bo_v[None, None, :]
    return outp


if __name__ == "__main__":
    rng = np.random.default_rng(0)
    demo = {
        "hidden_states": rng.standard_normal((B, S, HID), dtype=np.float32),
        "causal_mask": np.triu(np.ones((1, 1, S, S), np.float32), k=1),
        "padding_mask": np.zeros((B, S), np.float32),
        "Wq": (rng.standard_normal((HID, HID), dtype=np.float32) * 0.02),
        "bq": np.zeros((HID,), np.float32),
        "Wk": (rng.standard_normal((D, HID), dtype=np.float32) * 0.02),
        "bk": np.zeros((D,), np.float32),
        "Wv": (rng.standard_normal((D, HID), dtype=np.float32) * 0.02),
        "bv": np.zeros((D,), np.float32),
        "Wo": (rng.standard_normal((HID, HID), dtype=np.float32) * 0.02),
        "bo": np.zeros((HID,), np.float32),
    }
    o = kernel(**demo)
    print("kernel output", o.shape, o.dtype, float(np.abs(o).mean()))


# revision 19
# speedup vs baseline: 1.0160x; 1.0160x over previous
"""MQA (GQA with 1 KV group) attention kernel for 8 Trainium2 NeuronCores.

Sharding: core c -> batch b = c//4, head-group hg = c%4 (4 of 16 query heads).
Each core computes Q/K/V projections from x[b]^T, causal attention for its 4
heads in transposed layout (S^T[kv, q] tiles), and a partial output
projection out_partial = A_h @ Wo[:, cols_h]^T.  Host sums the 4 partials per
batch and adds bo.

v2 notes (vs the original): attention runs as two 2-head streams with PSUM
ping-pong (2 banks scores + 2 banks AV accum per stream = 8 banks total) and
a one-step software pipeline (AV of kt-1 issued with scores of kt) so the PE
never waits on the exp activation; softmax row-sums accumulate on the DVE in
bf16 and are broadcast-reduced by an all-ones matmul, then inverted with the
single-pass reciprocal_approx_fast; the causal mask is structural (upper
kv-tiles skipped, diagonal-band es tiles pre-zeroed and exp restricted to
valid columns, only the 128-wide triangle gets a mask multiply).
"""

import sys

sys.path.insert(0, "/opt/trn_rl_repo")

import ml_dtypes
import numpy as np

import concourse.bass as bass
import concourse.tile as tile
from concourse import bacc
from concourse import mybir
from concourse.bass import ts
from concourse.bass_utils import run_bass_kernel_spmd
from concourse.masks import make_identity

B, S, HID = 2, 2048, 2048
H, D = 16, 128
HPC = 4              # heads per core
DPH = HPC * D        # 512: head dims per core
NCORES = 8
SC = 512             # s-chunk (free dim for most matmuls)
NSC = S // SC        # 4
NT = S // 128        # 16 128-tiles along s / hid
NHT = HID // 128     # 16 hid tiles
SCALE = 1.0 / float(np.sqrt(D))
NEG = -1.0e9

F32 = mybir.dt.float32
BF16 = mybir.dt.bfloat16
NP_BF16 = ml_dtypes.bfloat16

_PROGRAM = None
LAST_RESULT = None


def _build_program():
    nc = bacc.Bacc()
    xT = nc.declare_dram_parameter("xT", [HID, S], BF16, isOutput=False)
    wq = nc.declare_dram_parameter("wq", [HID, DPH], BF16, isOutput=False)
    wk = nc.declare_dram_parameter("wk", [HID, D], BF16, isOutput=False)
    wv = nc.declare_dram_parameter("wv", [HID, D], BF16, isOutput=False)
    wo = nc.declare_dram_parameter("wo", [DPH, HID], BF16, isOutput=False)
    bq = nc.declare_dram_parameter("bq", [128, HPC], F32, isOutput=False)
    bkv = nc.declare_dram_parameter("bkv", [128, 2], F32, isOutput=False)
    padb = nc.declare_dram_parameter("padb", [128, NT], F32, isOutput=False)
    tri = nc.declare_dram_parameter("tri", [128, 2, 128], BF16, isOutput=False)
    out = nc.declare_dram_parameter("out", [S, HID], BF16, isOutput=True)

    Exp = mybir.ActivationFunctionType.Exp
    Ident = mybir.ActivationFunctionType.Identity

    with tile.TileContext(nc) as tc:
        with (
            tc.tile_pool(name="consts", bufs=1) as consts,
            tc.tile_pool(name="persist", bufs=1) as persist,
        ):
            # consts ride the GpSimd DGE so the Sync queue is free to start
            # streaming xT rows immediately
            ident = consts.tile([128, 128], BF16)
            make_identity(nc, ident[:])
            ones128 = consts.tile([128, 128], BF16)
            nc.vector.memset(ones128[:], 1.0)
            bq_sb = consts.tile([128, HPC], F32)
            nc.gpsimd.dma_start(bq_sb[:], bq[:])
            bkv_sb = consts.tile([128, 2], F32)
            nc.gpsimd.dma_start(bkv_sb[:], bkv[:])
            padb_sb = consts.tile([128, NT], F32)
            nc.gpsimd.dma_start(padb_sb[:], padb[:])
            tri_sb = consts.tile([128, 2, 128], BF16)
            nc.gpsimd.dma_start(tri_sb[:], tri[:])

            # Persistent activations (live across stages)
            wo_sb = persist.tile([128, HPC, HID], BF16)  # stage-3 weights
            QT = persist.tile([128, HPC, S], BF16)   # Q^T per head: [d, h, q]
            KT = persist.tile([128, S], BF16)        # K^T: [d, kv]
            V = persist.tile([128, NT, 128], BF16)   # V tiles: [kv_p, kv_tile, d]
            OT = persist.tile([128, HPC, S], BF16)   # normalized (exp(S) V)^T

            # Diagonal-band es tiles: columns left of the diagonal stay zero
            # forever (exp never writes them), so AV/rowsum reads are exact.
            esd = [
                [
                    persist.tile([128, 2, SC], BF16, name=f"esd_{j}_{s}")
                    for s in range(2)
                ]
                for j in range(4)
            ]
            for j in range(4):
                for s in range(2):
                    nc.vector.memset(esd[j][s][:], 0.0)

            # ---------------- Stage 1: projections ----------------
            # Pass A streams K and V matmuls row-by-row as xT rows land
            # (8 MMs per row, all 8 PSUM banks = K/V x 4 s-chunks), so the
            # PE starts as soon as the first row arrives instead of waiting
            # for the whole 8 MiB of x.  Pass B (x resident) does the V
            # transposes and Q chains.
            with (
                tc.tile_pool(name="w1", bufs=1) as w1p,
                tc.tile_pool(name="xr", bufs=1) as xrp,
                tc.tile_pool(name="vt", bufs=4) as vtp,
            ):
                # x^T resident: one whole-row DMA per 128-partition tile
                # (4 KiB/partition contiguous descriptors) on the Sync DGE;
                # weights ride the Scalar DGE in parallel.
                xres = []
                for ht in range(NHT):
                    xr_t = xrp.tile([128, S], BF16, tag=f"x{ht}",
                                    name=f"xr_{ht}")
                    nc.sync.dma_start(xr_t[:], xT[ts(ht, 128), :])
                    xres.append(xr_t)
                wk_sb = w1p.tile([128, NHT, D], BF16)
                nc.scalar.dma_start(
                    wk_sb[:], wk.rearrange("(t p) d -> p t d", p=128)
                )
                wv_sb = w1p.tile([128, NHT, D], BF16)
                nc.scalar.dma_start(
                    wv_sb[:], wv.rearrange("(t p) d -> p t d", p=128)
                )
                wq_sb = w1p.tile([128, NHT, DPH], BF16)
                nc.scalar.dma_start(
                    wq_sb[:], wq.rearrange("(t p) d -> p t d", p=128)
                )

                vts = []
                with tc.tile_pool(name="ps1", bufs=1, space="PSUM") as ps1:
                    psks = [
                        ps1.tile([128, SC], F32, tag=f"k{sc}", name=f"psk{sc}")
                        for sc in range(NSC)
                    ]
                    psvs = [
                        ps1.tile([128, SC], F32, tag=f"v{sc}", name=f"psv{sc}")
                        for sc in range(NSC)
                    ]
                    for ht in range(NHT):
                        for sc in range(NSC):
                            nc.tensor.matmul(
                                psks[sc][:], wk_sb[:, ht, :],
                                xres[ht][:, ts(sc, SC)],
                                start=(ht == 0), stop=(ht == NHT - 1),
                            )
                        for sc in range(NSC):
                            nc.tensor.matmul(
                                psvs[sc][:], wv_sb[:, ht, :],
                                xres[ht][:, ts(sc, SC)],
                                start=(ht == 0), stop=(ht == NHT - 1),
                            )
                    for sc in range(NSC):
                        nc.scalar.activation(
                            KT[:, ts(sc, SC)], psks[sc][:], Ident,
                            bias=bkv_sb[:, 0:1],
                        )
                        vt_s = vtp.tile([128, SC], BF16, tag=f"vt{sc}")
                        nc.scalar.activation(
                            vt_s[:], psvs[sc][:], Ident, bias=bkv_sb[:, 1:2]
                        )
                        vts.append(vt_s)

                with (
                    tc.tile_pool(name="psQ", bufs=1, space="PSUM") as psQ,
                    tc.tile_pool(name="pstr", bufs=2, space="PSUM") as pstr,
                ):
                    for sc in range(NSC):
                        for j in range(SC // 128):
                            pst = pstr.tile([128, 128], BF16, tag="tr")
                            nc.tensor.transpose(
                                pst[:], vts[sc][:, ts(j, 128)], ident[:]
                            )
                            nc.scalar.copy(V[:, sc * 4 + j, :], pst[:])
                    # Q^T chunks (4 heads x 4 s-chunks, 6-bank rotation)
                    for sc in range(NSC):
                        for dt in range(HPC):
                            psq = psQ.tile(
                                [128, SC], F32, tag=f"q{(sc * HPC + dt) % 6}",
                                name=f"psq_{sc}_{dt}",
                            )
                            for ht in range(NHT):
                                nc.tensor.matmul(
                                    psq[:], wq_sb[:, ht, ts(dt, 128)],
                                    xres[ht][:, ts(sc, SC)],
                                    start=(ht == 0), stop=(ht == NHT - 1),
                                )
                            nc.scalar.activation(
                                QT[:, dt, ts(sc, SC)], psq[:], Ident,
                                bias=bq_sb[:, dt : dt + 1],
                            )

            # ---------------- Stage 2: attention ----------------
            # Two 2-head streams s=0 (heads 0,1) and s=1 (heads 2,3).
            with (
                tc.tile_pool(name="es", bufs=6) as esp,
                tc.tile_pool(name="acc", bufs=2) as accp,
                tc.tile_pool(name="rb", bufs=2) as rbp,
                tc.tile_pool(name="psSa", bufs=1, space="PSUM") as psSa,
                tc.tile_pool(name="psSb", bufs=1, space="PSUM") as psSb,
                tc.tile_pool(name="psOa", bufs=1, space="PSUM") as psOa,
                tc.tile_pool(name="psOb", bufs=1, space="PSUM") as psOb,
            ):
                # prefetch the stage-3 weights while the DMA queue is idle
                nc.sync.dma_start(
                    wo_sb[:], wo.rearrange("(t p) d -> p t d", p=128)
                )
                psS_pool = [psSa, psSb]
                psO_pool = [psOa, psOb]
                pending_fin = None

                def emit_finalize():
                    # normalize: rowsum broadcast via all-ones matmul,
                    # single-pass approx reciprocal, multiply into OT
                    nonlocal pending_fin
                    if pending_fin is None:
                        return
                    fqc, fpsos, faccs = pending_fin
                    pending_fin = None
                    for s in range(2):
                        psb = psS_pool[s].tile(
                            [128, 2, SC], F32, tag="s", name=f"psb_{s}"
                        )
                        for h in range(2):
                            nc.tensor.matmul(
                                psb[:, h, :], ones128[:], faccs[s][:, h, :],
                                start=True, stop=True,
                            )
                        rb = rbp.tile([128, 2, SC], F32, tag=f"r{s}")
                        nc.vector.reciprocal_approx_fast(rb[:], psb[:])
                        nc.vector.tensor_mul(
                            OT[:, 2 * s : 2 * s + 2, ts(fqc, SC)],
                            fpsos[s][:], rb[:],
                        )

                for qc in range(NSC):
                    nkt = 4 * qc + 4
                    psos = [
                        psO_pool[s].tile(
                            [128, 2, SC], F32, tag="o", name=f"pso_{s}"
                        )
                        for s in range(2)
                    ]
                    accs = [
                        accp.tile(
                            [128, 2, SC], BF16, tag=f"a{s}", name=f"acc_{s}"
                        )
                        for s in range(2)
                    ]
                    es_hist = {}

                    def emit_av(kt, psos=psos, es_hist=es_hist, nkt=nkt,
                                qc=qc):
                        j = kt - 4 * qc
                        lo = 128 * j if j >= 0 else 0
                        for s in range(2):
                            es_t = es_hist[(kt, s)]
                            for h in range(2):
                                nc.tensor.matmul(
                                    psos[s][:, h, lo:SC], V[:, kt, :],
                                    es_t[:, h, lo:SC],
                                    start=(kt == 0), stop=(kt == nkt - 1),
                                    skip_group_check=(j > 0),
                                )

                    for kt in range(nkt):
                        j = kt - 4 * qc
                        lo = 128 * j if j >= 0 else 0
                        # scores matmuls (diagonal kts: only valid columns)
                        pss = [
                            psS_pool[s].tile(
                                [128, 2, SC], F32, tag="s", name=f"pss_{s}"
                            )
                            for s in range(2)
                        ]
                        for s in range(2):
                            for h in range(2):
                                nc.tensor.matmul(
                                    pss[s][:, h, lo:SC], KT[:, ts(kt, 128)],
                                    QT[:, 2 * s + h,
                                       qc * SC + lo : (qc + 1) * SC],
                                    start=True, stop=True,
                                )
                        # exp (+ padding bias); diagonal kts write only the
                        # valid column range of the pre-zeroed esd tiles
                        for s in range(2):
                            if j >= 0:
                                es_t = esd[j][s]
                                nc.scalar.activation(
                                    es_t[:, :, lo:SC], pss[s][:, :, lo:SC],
                                    Exp, bias=padb_sb[:, kt : kt + 1],
                                    scale=SCALE,
                                )
                            else:
                                es_t = esp.tile(
                                    [128, 2, SC], BF16, tag=f"e{s}",
                                    name=f"es_{s}",
                                )
                                nc.scalar.activation(
                                    es_t[:], pss[s][:], Exp,
                                    bias=padb_sb[:, kt : kt + 1], scale=SCALE,
                                )
                            es_hist[(kt, s)] = es_t
                        # triangle mask on the 128-wide diagonal block
                        if j >= 0:
                            for s in range(2):
                                es_t = es_hist[(kt, s)]
                                nc.vector.tensor_mul(
                                    es_t[:, :, lo : lo + 128],
                                    es_t[:, :, lo : lo + 128],
                                    tri_sb[:],
                                )
                        # row-sum accumulation (bf16, 2x DVE rate)
                        for s in range(2):
                            es_t = es_hist[(kt, s)]
                            if kt == 0:
                                nc.vector.tensor_scalar_mul(
                                    accs[s][:], es_t[:], 1.0
                                )
                            else:
                                nc.vector.tensor_add(
                                    accs[s][:], accs[s][:], es_t[:]
                                )
                        # AV of previous kt (software pipeline: PE never
                        # waits on this kt's exp); previous qc's finalize
                        # slots in after the first scores of this qc
                        if kt > 0:
                            emit_av(kt - 1)
                        if kt == 0:
                            emit_finalize()
                    emit_av(nkt - 1)
                    pending_fin = (qc, psos, accs)
                emit_finalize()

            # ---------------- Stage 3: output projection ----------------
            with (
                tc.tile_pool(name="outsb", bufs=4) as outp,
                tc.tile_pool(name="ps3", bufs=2, space="PSUM") as ps3,
            ):
                for st in range(NT):
                    psT = ps3.tile([128, HID // SC, SC], F32, tag="c")
                    for dt in range(HPC):
                        for hc in range(HID // SC):
                            nc.tensor.matmul(
                                psT[:, hc, :],
                                OT[:, dt, ts(st, 128)],
                                wo_sb[:, dt, ts(hc, SC)],
                                start=(dt == 0), stop=(dt == HPC - 1),
                            )
                    # evacuate: half on ACT, half on DVE; 4 output DMAs
                    o1 = outp.tile([128, 2, SC], BF16, tag="o1")
                    nc.scalar.copy(o1[:], psT[:, 0:2, :])
                    o2 = outp.tile([128, 2, SC], BF16, tag="o2")
                    nc.vector.tensor_scalar_mul(o2[:], psT[:, 2:4, :], 1.0)
                    for hc in range(2):
                        nc.sync.dma_start(
                            out[ts(st, 128), ts(hc, SC)], o1[:, hc, :]
                        )
                        nc.sync.dma_start(
                            out[ts(st, 128), ts(hc + 2, SC)], o2[:, hc, :]
                        )
    nc.compile()
    return nc


def _get_program():
    global _PROGRAM
    if _PROGRAM is None:
        _PROGRAM = _build_program()
    return _PROGRAM


def kernel(**inputs):
    global LAST_RESULT
    hs = np.ascontiguousarray(inputs["hidden_states"], dtype=np.float32)
    pad = np.ascontiguousarray(inputs["padding_mask"], dtype=np.float32)
    Wq = np.asarray(inputs["Wq"], dtype=np.float32)
    Wk = np.asarray(inputs["Wk"], dtype=np.float32)
    Wv = np.asarray(inputs["Wv"], dtype=np.float32)
    Wo = np.asarray(inputs["Wo"], dtype=np.float32)
    bq_v = np.asarray(inputs["bq"], dtype=np.float32)
    bk_v = np.asarray(inputs["bk"], dtype=np.float32)
    bv_v = np.asarray(inputs["bv"], dtype=np.float32)
    bo_v = np.asarray(inputs["bo"], dtype=np.float32)

    xTs = [np.ascontiguousarray(hs[b].T).astype(NP_BF16) for b in range(B)]
    WqT = Wq.T  # [HID, HID]
    WkT = np.ascontiguousarray(Wk.T).astype(NP_BF16)  # [HID, D]
    WvT = np.ascontiguousarray(Wv.T).astype(NP_BF16)
    WoT = Wo.T  # [HID, HID]

    # triangle mask for the 128-wide diagonal block: tri[p, q] = 1 if p <= q
    p_i = np.arange(128)[:, None]
    f_i = np.arange(128)[None, :]
    tri1 = (p_i <= f_i).astype(np.float32)
    tri = np.ascontiguousarray(
        np.broadcast_to(tri1[:, None, :], (128, 2, 128))
    ).astype(NP_BF16)

    padbs = [
        np.ascontiguousarray((NEG * pad[b]).reshape(NT, 128).T) for b in range(B)
    ]
    bqs = [
        np.ascontiguousarray(
            bq_v[hg * DPH : (hg + 1) * DPH].reshape(HPC, 128).T
        )
        for hg in range(HPC)
    ]
    bkv = np.ascontiguousarray(np.stack([bk_v, bv_v], axis=1))  # [128, 2]

    nc = _get_program()
    in_maps = []
    for c in range(NCORES):
        b, hg = c // 4, c % 4
        in_maps.append(
            {
                "xT": xTs[b],
                "wq": np.ascontiguousarray(
                    WqT[:, hg * DPH : (hg + 1) * DPH]
                ).astype(NP_BF16),
                "wk": WkT,
                "wv": WvT,
                "wo": np.ascontiguousarray(
                    WoT[hg * DPH : (hg + 1) * DPH, :]
                ).astype(NP_BF16),
                "bq": bqs[hg],
                "bkv": bkv,
                "padb": padbs[b],
                "tri": tri,
            }
        )

    LAST_RESULT = run_bass_kernel_spmd(nc, in_maps, list(range(NCORES)))
    res = LAST_RESULT.results

    outp = np.zeros((B, S, HID), np.float32)
    for c in range(NCORES):
        outp[c // 4] += res[c]["out"]
    outp += bo_v[None, None, :]
    return outp


if __name__ == "__main__":
    rng = np.random.default_rng(0)
    demo = {
        "hidden_states": rng.standard_normal((B, S, HID), dtype=np.float32),
        "causal_mask": np.triu(np.ones((1, 1, S, S), np.float32), k=1),
        "padding_mask": np.zeros((B, S), np.float32),
        "Wq": (rng.standard_normal((HID, HID), dtype=np.float32) * 0.02),
        "bq": np.zeros((HID,), np.float32),
        "Wk": (rng.standard_normal((D, HID), dtype=np.float32) * 0.02),
        "bk": np.zeros((D,), np.float32),
        "Wv": (rng.standard_normal((D, HID), dtype=np.float32) * 0.02),
        "bv": np.zeros((D,), np.float32),
        "Wo": (rng.standard_normal((HID, HID), dtype=np.float32) * 0.02),
        "bo": np.zeros((HID,), np.float32),
    }
    o = kernel(**demo)
    print("kernel output", o.shape, o.dtype, float(np.abs(o).mean()))
